# revision 9
# baseline (speedup 1.0000x reference)
"""Trainium2 Bass kernel for a Direct-Form-II-transposed IIR single-step update.

reference semantics (all fp32):
    out  = input * b[0] + v[..., 0]                  # [B, C]
    v_new[..., m] = input * b[m+1] - out * a[m]      # m = 0..7
    v_new[..., m] += v[..., m+1]   for m < 7
returns (out [B, C], v_new [B, C, 8])

Sharding: data-parallel over dim 0 (batch) across 8 NeuronCores.
Per core: N = 32*32768 = 1,048,576 rows, processed as 8 tiles of
[128 partitions x 1024 rows]. v rows stay in their natural interleaved
[row, 8] HBM layout; on-chip columns are accessed with stride-8 APs.
"""

from contextlib import ExitStack

import numpy as np

import concourse.bass as bass
import concourse.tile as tile
from concourse import bacc, mybir
from concourse.bass_utils import run_bass_kernel_spmd

NCORES = 8
B, C, M = 256, 32768, 8
BP = B // NCORES          # batch rows per core
N = BP * C                # rows per core
P = 128                   # SBUF partitions
F = 1024                  # rows per partition per tile
T = N // (P * F)          # tiles per core

F32 = mybir.dt.float32

_cached = None


def _build():
    """Build the single-core Bass program (same program runs SPMD on 8 cores)."""
    nc = bacc.Bacc(
        "TRN2",
        target_bir_lowering=False,
        debug=False,
        enable_asserts=False,
    )

    x_d = nc.dram_tensor("x", [T, P, F], F32, kind="ExternalInput").ap()
    v_d = nc.dram_tensor("v", [T, P, F * M], F32, kind="ExternalInput").ap()
    b_d = nc.dram_tensor("b", [1, M + 1], F32, kind="ExternalInput").ap()
    a_d = nc.dram_tensor("a", [1, M], F32, kind="ExternalInput").ap()
    o_d = nc.dram_tensor("o", [T, P, F], F32, kind="ExternalOutput").ap()
    vn_d = nc.dram_tensor("vn", [T, P, F * M], F32, kind="ExternalOutput").ap()

    mult = mybir.AluOpType.mult
    add = mybir.AluOpType.add

    with tile.TileContext(nc) as tc, ExitStack() as ctx:
        cpool = ctx.enter_context(tc.tile_pool(name="coef", bufs=1))
        vpool = ctx.enter_context(tc.tile_pool(name="vin", bufs=2))
        xpool = ctx.enter_context(tc.tile_pool(name="xin", bufs=2))
        vnpool = ctx.enter_context(tc.tile_pool(name="vout", bufs=2))
        opool = ctx.enter_context(tc.tile_pool(name="oout", bufs=2))
        upool = ctx.enter_context(tc.tile_pool(name="utmp", bufs=3))

        # --- coefficient prep (one-time) -------------------------------
        row = cpool.tile([1, 2 * M + 1], F32)
        nc.sync.dma_start(row[:, 0 : M + 1], b_d[:])
        nc.sync.dma_start(row[:, M + 1 : 2 * M + 1], a_d[:])
        rep = cpool.tile([P, 2 * M + 1], F32)
        nc.gpsimd.partition_broadcast(rep[:], row[:])
        na = cpool.tile([P, M], F32)  # -a, replicated per partition
        nc.vector.tensor_scalar_mul(na[:], rep[:, M + 1 : 2 * M + 1], -1.0)
        b0r = rep[:, 0:1]
        brep = rep  # brep[:, 1+m : 2+m] is b[m+1] as a per-partition scalar

        # --- main loop -------------------------------------------------
        for t in range(T):
            vt = vpool.tile([P, F * M], F32)
            nc.sync.dma_start(vt[:], v_d[t])
            xt = xpool.tile([P, F], F32)
            nc.sync.dma_start(xt[:], x_d[t])

            v3 = vt[:].rearrange("p (f m) -> p f m", m=M)
            vnt = vnpool.tile([P, F * M], F32)
            vn3 = vnt[:].rearrange("p (f m) -> p f m", m=M)
            ot = opool.tile([P, F], F32)

            v0 = v3[:, :, 0]
            # out = x*b0 + v0
            nc.vector.scalar_tensor_tensor(ot[:], xt[:], b0r, v0, mult, add)
            for m in range(M - 1):
                u = upool.tile([P, F], F32)
                # u = -a[m]*out + v[:, m+1]
                nc.vector.scalar_tensor_tensor(
                    u[:], ot[:], na[:, m : m + 1], v3[:, :, m + 1], mult, add
                )
                # vn[:, m] = b[m+1]*x + u
                nc.vector.scalar_tensor_tensor(
                    vn3[:, :, m], xt[:], brep[:, m + 1 : m + 2], u[:], mult, add
                )
            # m = 7: no shift-in term
            u = upool.tile([P, F], F32)
            nc.vector.tensor_scalar_mul(u[:], ot[:], na[:, M - 1 : M])
            nc.vector.scalar_tensor_tensor(
                vn3[:, :, M - 1], xt[:], brep[:, M : M + 1], u[:], mult, add
            )

            # stores on the ACT HWDGE ring (loads use the SP ring)
            nc.scalar.dma_start(o_d[t], ot[:])
            nc.scalar.dma_start(vn_d[t], vnt[:])

    nc.finalize()  # Bacc compile pipeline: event sems, reg alloc, lib loads
    return nc


def _get_nc():
    global _cached
    if _cached is None:
        _cached = _build()
    return _cached


def _run(input, v, b, a, trace=False, **spmd_kwargs):
    nc = _get_nc()

    x = np.ascontiguousarray(np.asarray(input, dtype=np.float32)).reshape(B, C)
    vv = np.ascontiguousarray(np.asarray(v, dtype=np.float32))
    bb = np.ascontiguousarray(np.asarray(b, dtype=np.float32)).reshape(1, M + 1)
    aa = np.ascontiguousarray(np.asarray(a, dtype=np.float32)).reshape(1, M)

    in_maps = []
    for c in range(NCORES):
        xs = x[c * BP : (c + 1) * BP].reshape(T, P, F)
        vs = vv[c * BP : (c + 1) * BP].reshape(T, P, F * M)
        in_maps.append({"x": xs, "v": vs, "b": bb, "a": aa})

    res = run_bass_kernel_spmd(
        nc, in_maps, list(range(NCORES)), trace=trace, **spmd_kwargs
    )

    out = np.empty((B, C), dtype=np.float32)
    v_new = np.empty((B, C, M), dtype=np.float32)
    for c in range(NCORES):
        out[c * BP : (c + 1) * BP] = res.results[c]["o"].reshape(BP, C)
        v_new[c * BP : (c + 1) * BP] = res.results[c]["vn"].reshape(BP, C, M)
    return (out, v_new), res


def kernel(input, v, b, a):
    (out, v_new), _ = _run(input, v, b, a)
    return out, v_new


# revision 13
# speedup vs baseline: 1.1926x; 1.1926x over previous
"""Trainium2 Bass kernel for a Direct-Form-II-transposed IIR single-step update.

reference semantics (all fp32):
    out  = input * b[0] + v[..., 0]                  # [B, C]
    v_new[..., m] = input * b[m+1] - out * a[m]      # m = 0..7
    v_new[..., m] += v[..., m+1]   for m < 7
returns (out [B, C], v_new [B, C, 8])

Sharding: data-parallel over dim 0 (batch) across 8 NeuronCores.
Per core: N = 32*32768 = 1,048,576 rows, processed as 8 tiles of
[128 partitions x 1024 rows]. v rows stay in their natural interleaved
[row, 8] HBM layout; on-chip columns are accessed with stride-8 APs.
"""

from contextlib import ExitStack

import numpy as np

import concourse.bass as bass
import concourse.tile as tile
from concourse import bacc, mybir
from concourse.bass_utils import run_bass_kernel_spmd

NCORES = 8
B, C, M = 256, 32768, 8
BP = B // NCORES          # batch rows per core
N = BP * C                # rows per core
P = 128                   # SBUF partitions
F = 1024                  # rows per partition per tile
T = N // (P * F)          # tiles per core

F32 = mybir.dt.float32

_cached = None


def _build():
    """Build the single-core Bass program (same program runs SPMD on 8 cores)."""
    nc = bacc.Bacc(
        "TRN2",
        target_bir_lowering=False,
        debug=False,
        enable_asserts=False,
    )

    x_d = nc.dram_tensor("x", [T, P, F], F32, kind="ExternalInput").ap()
    v_d = nc.dram_tensor("v", [T, P, F * M], F32, kind="ExternalInput").ap()
    b_d = nc.dram_tensor("b", [1, M + 1], F32, kind="ExternalInput").ap()
    a_d = nc.dram_tensor("a", [1, M], F32, kind="ExternalInput").ap()
    o_d = nc.dram_tensor("o", [T, P, F], F32, kind="ExternalOutput").ap()
    vn_d = nc.dram_tensor("vn", [T, P, F * M], F32, kind="ExternalOutput").ap()

    mult = mybir.AluOpType.mult
    add = mybir.AluOpType.add
    Copy = mybir.ActivationFunctionType.Copy

    with tile.TileContext(nc) as tc, ExitStack() as ctx:
        cpool = ctx.enter_context(tc.tile_pool(name="coef", bufs=1))
        vpool = ctx.enter_context(tc.tile_pool(name="vin", bufs=2))
        xpool = ctx.enter_context(tc.tile_pool(name="xin", bufs=2))
        vnpool = ctx.enter_context(tc.tile_pool(name="vout", bufs=2))
        opool = ctx.enter_context(tc.tile_pool(name="oout", bufs=2))
        tmpool = ctx.enter_context(tc.tile_pool(name="ttmp", bufs=4))

        # --- coefficient prep (one-time) -------------------------------
        row = cpool.tile([1, 2 * M + 1], F32)
        nc.sync.dma_start(row[:, 0 : M + 1], b_d[:])
        nc.sync.dma_start(row[:, M + 1 : 2 * M + 1], a_d[:])
        rep = cpool.tile([P, 2 * M + 1], F32)
        nc.gpsimd.partition_broadcast(rep[:], row[:])
        na = cpool.tile([P, M], F32)  # -a, replicated per partition
        nc.vector.tensor_scalar_mul(na[:], rep[:, M + 1 : 2 * M + 1], -1.0)
        b0r = rep[:, 0:1]
        brep = rep  # brep[:, 1+m : 2+m] is b[m+1] as a per-partition scalar

        # --- main loop -------------------------------------------------
        for t in range(T):
            vt = vpool.tile([P, F * M], F32)
            nc.sync.dma_start(vt[:], v_d[t])
            xt = xpool.tile([P, F], F32)
            nc.sync.dma_start(xt[:], x_d[t])

            v3 = vt[:].rearrange("p (f m) -> p f m", m=M)
            vnt = vnpool.tile([P, F * M], F32)
            vn3 = vnt[:].rearrange("p (f m) -> p f m", m=M)
            ot = opool.tile([P, F], F32)

            v0 = v3[:, :, 0]
            # out = x*b0 + v0
            nc.vector.scalar_tensor_tensor(ot[:], xt[:], b0r, v0, mult, add)
            for m in range(M):
                # tm = b[m+1]*x on the (otherwise idle) scalar engine
                tm = tmpool.tile([P, F], F32)
                nc.scalar.activation(
                    tm[:], xt[:], Copy, bias=0.0, scale=brep[:, m + 1 : m + 2]
                )
                # vn[:, m] = -a[m]*out + tm
                nc.vector.scalar_tensor_tensor(
                    vn3[:, :, m], ot[:], na[:, m : m + 1], tm[:], mult, add
                )
            # one inner-unit shift-add replaces 7 strided column adds:
            # vn[:, :, 0:7] += v[:, :, 1:8]
            nc.vector.tensor_add(vn3[:, :, 0 : M - 1], vn3[:, :, 0 : M - 1], v3[:, :, 1:M])

            # stores on the ACT HWDGE ring (loads use the SP ring)
            nc.scalar.dma_start(o_d[t], ot[:])
            nc.scalar.dma_start(vn_d[t], vnt[:])

    nc.finalize()  # Bacc compile pipeline: event sems, reg alloc, lib loads
    return nc


def _get_nc():
    global _cached
    if _cached is None:
        _cached = _build()
    return _cached


def _run(input, v, b, a, trace=False, **spmd_kwargs):
    nc = _get_nc()

    x = np.ascontiguousarray(np.asarray(input, dtype=np.float32)).reshape(B, C)
    vv = np.ascontiguousarray(np.asarray(v, dtype=np.float32))
    bb = np.ascontiguousarray(np.asarray(b, dtype=np.float32)).reshape(1, M + 1)
    aa = np.ascontiguousarray(np.asarray(a, dtype=np.float32)).reshape(1, M)

    in_maps = []
    for c in range(NCORES):
        xs = x[c * BP : (c + 1) * BP].reshape(T, P, F)
        vs = vv[c * BP : (c + 1) * BP].reshape(T, P, F * M)
        in_maps.append({"x": xs, "v": vs, "b": bb, "a": aa})

    res = run_bass_kernel_spmd(
        nc, in_maps, list(range(NCORES)), trace=trace, **spmd_kwargs
    )

    out = np.empty((B, C), dtype=np.float32)
    v_new = np.empty((B, C, M), dtype=np.float32)
    for c in range(NCORES):
        out[c * BP : (c + 1) * BP] = res.results[c]["o"].reshape(BP, C)
        v_new[c * BP : (c + 1) * BP] = res.results[c]["vn"].reshape(BP, C, M)
    return (out, v_new), res


def kernel(input, v, b, a):
    (out, v_new), _ = _run(input, v, b, a)
    return out, v_new
